# revision 23
# baseline (speedup 1.0000x reference)
"""Trainium2 Bass kernel for nn_AttentionModule (B=8, C=256, L=2048, D=32).

Per-batch computation (data-parallel: one batch per NeuronCore, 8 cores):
    qT = Wq @ x + bq            # (D, L)
    kT = Wk @ x + bk            # (D, L)
    vT = x.T @ Wv.T + bv        # (L, C)   -- v transposed, computed directly
    ST = kT.T @ qT              # (L_j, L_i) = S[i,j] transposed
    E  = exp(ST)                # no max-subtraction: max|S| ~ 46, exp fits fp32
    Z  = sum_j E[j, i]          # DVE accumulate; partition-reduce on the PE
    U  = vT.T @ E               # (C, L_i)
    y  = gamma * U / Z + x

Engine split: PE does all matmuls incl. Z partition-reduction, the Z
transpose (so the reciprocal runs 128-wide instead of on one lane), and
the 1/Z broadcast. ACT does exp + q/k copyback. DVE does Z accumulation,
vT copyback, U copyback, reciprocal. GPSIMD does the final
y = U*(gamma/Z) + x elementwise work. Score matmuls (K=32) are 4x
row-packed via tile_position. Weights are transposed / replicated / cast
host-side.
"""

import numpy as np

B, C, L, D = 8, 256, 2048, 32
NCORES = 8

_cache = {}


def _build_nc():
    from contextlib import ExitStack

    import concourse.bacc as bacc
    import concourse.tile as tile
    from concourse import mybir

    f32 = mybir.dt.float32
    bf16 = mybir.dt.bfloat16
    EXP = mybir.ActivationFunctionType.Exp
    IDENT = mybir.ActivationFunctionType.Identity

    nc = bacc.Bacc("TRN2", target_bir_lowering=False, debug=False)

    x_d = nc.dram_tensor("x", [C, L], f32, kind="ExternalInput")
    xb_d = nc.dram_tensor("xb", [C, L], bf16, kind="ExternalInput")
    wqk_d = nc.dram_tensor("wqk", [C, 2 * D], bf16, kind="ExternalInput")
    wvT_d = nc.dram_tensor("wvT", [C, C], bf16, kind="ExternalInput")
    bqk_d = nc.dram_tensor("bqk", [2 * D, 1], f32, kind="ExternalInput")
    bvr_d = nc.dram_tensor("bvr", [128, C], f32, kind="ExternalInput")
    gammac_d = nc.dram_tensor("gammac", [128, 1], f32, kind="ExternalInput")
    ones_d = nc.dram_tensor("ones", [128, 1], bf16, kind="ExternalInput")
    onesr_d = nc.dram_tensor("onesr", [1, 128], f32, kind="ExternalInput")
    ident_d = nc.dram_tensor("ident", [128, 128], f32, kind="ExternalInput")
    y_d = nc.dram_tensor("y", [C, L], f32, kind="ExternalOutput")

    x_ap = x_d.ap()
    y_ap = y_d.ap()

    with tile.TileContext(nc) as tc, ExitStack() as ctx:
        singles = ctx.enter_context(tc.tile_pool(name="singles", bufs=1))
        big = ctx.enter_context(tc.tile_pool(name="big", bufs=1))
        ps = ctx.enter_context(tc.tile_pool(name="ps", bufs=2, space="PSUM"))
        up = ctx.enter_context(tc.tile_pool(name="up", bufs=1, space="PSUM"))
        zp = ctx.enter_context(tc.tile_pool(name="zp", bufs=2, space="PSUM"))
        epool = ctx.enter_context(tc.tile_pool(name="epool", bufs=4))
        ypool = ctx.enter_context(tc.tile_pool(name="ypool", bufs=4))
        uspool = ctx.enter_context(tc.tile_pool(name="uspool", bufs=2))
        rpool = ctx.enter_context(tc.tile_pool(name="rpool", bufs=2))

        # ---- load compute inputs first (xb then weights); fp32 x comes later ----
        xb_sb = []
        for ct in range(2):
            tb = big.tile([128, L], bf16, tag=f"xb{ct}")
            nc.sync.dma_start(out=tb[:], in_=xb_d.ap()[ct * 128:(ct + 1) * 128, :])
            xb_sb.append(tb)
        wqk_sb, wvT_sb = [], []
        for ct in range(2):
            tq = singles.tile([128, 2 * D], bf16, tag=f"wqk{ct}")
            nc.sync.dma_start(out=tq[:], in_=wqk_d.ap()[ct * 128:(ct + 1) * 128, :])
            wqk_sb.append(tq)
            tv = singles.tile([128, C], bf16, tag=f"wv{ct}")
            nc.sync.dma_start(out=tv[:], in_=wvT_d.ap()[ct * 128:(ct + 1) * 128, :])
            wvT_sb.append(tv)
        bqk_sb = singles.tile([2 * D, 1], f32, tag="bqk")
        nc.sync.dma_start(out=bqk_sb[:], in_=bqk_d.ap()[:, :])
        bvr_sb = singles.tile([128, C], f32, tag="bvr")
        nc.sync.dma_start(out=bvr_sb[:], in_=bvr_d.ap()[:, :])
        gammac_sb = singles.tile([128, 1], f32, tag="gammac")
        nc.sync.dma_start(out=gammac_sb[:], in_=gammac_d.ap()[:, :])
        ones_sb = singles.tile([128, 1], bf16, tag="ones")
        nc.sync.dma_start(out=ones_sb[:], in_=ones_d.ap()[:, :])
        onesr_sb = singles.tile([1, 128], f32, tag="onesr")
        nc.sync.dma_start(out=onesr_sb[:], in_=onesr_d.ap()[:, :])
        ident_sb = singles.tile([128, 128], f32, tag="ident")
        nc.sync.dma_start(out=ident_sb[:], in_=ident_d.ap()[:, :])

        # ---- projections ----
        # qT4: qT[d, i] replicated across the four 32-partition strips so the
        # row-packed score matmuls can stream the moving operand from any strip.
        qT4 = big.tile([128, L], bf16, tag="qT4")
        kT64 = big.tile([2 * D, L], bf16, tag="kT64")
        for it in range(4):
            p = ps.tile([128, 1024], f32, tag="ps")
            for ct in range(2):
                nc.tensor.matmul(
                    p[:2 * D, 0:512],
                    lhsT=wqk_sb[ct][:],
                    rhs=xb_sb[ct][:, it * 512:(it + 1) * 512],
                    start=(ct == 0),
                    stop=(ct == 1),
                )
            nc.scalar.activation(
                qT4[0:D, it * 512:(it + 1) * 512], p[0:D, 0:512], IDENT,
                bias=bqk_sb[0:D, :],
            )
            nc.scalar.activation(
                kT64[D:2 * D, it * 512:(it + 1) * 512], p[D:2 * D, 0:512], IDENT,
                bias=bqk_sb[D:2 * D, :],
            )
        for g in range(1, 4):
            nc.sync.dma_start(out=qT4[32 * g:32 * (g + 1), :], in_=qT4[0:D, :])
        # kT4: strip g holds kT j-blocks {4J+g}; round J lives at free cols J*128.
        kT4 = big.tile([128, 512], bf16, tag="kT4")
        for g in range(4):
            nc.sync.dma_start(
                out=kT4[32 * g:32 * (g + 1), :].rearrange("d (J j) -> d J j", j=128),
                in_=kT64[D:2 * D, :].rearrange("d (J G j) -> d J G j", G=4, j=128)[:, :, g, :],
            )

        # vT[j, c] stored as [128, 16*256]: block jb holds vT[jb*128 + p, c]
        vT_sb = big.tile([128, 16 * C], bf16, tag="vT")
        for lb in range(16):
            p = ps.tile([128, 1024], f32, tag="ps")
            for ct in range(2):
                nc.tensor.matmul(
                    p[:, :C],
                    lhsT=xb_sb[ct][:, lb * 128:(lb + 1) * 128],
                    rhs=wvT_sb[ct][:],
                    start=(ct == 0),
                    stop=(ct == 1),
                )
            nc.vector.tensor_add(
                vT_sb[:, lb * C:(lb + 1) * C], p[:, :C], bvr_sb[:]
            )

        # fp32 x, needed only for the residual at the end
        x_sb = []
        for ct in range(2):
            t = big.tile([128, L], f32, tag=f"x{ct}")
            nc.sync.dma_start(out=t[:], in_=x_ap[ct * 128:(ct + 1) * 128, :])
            x_sb.append(t)

        # ---- attention, processed in i-quarters of 512 columns ----
        # emit_tail: Z partition-reduce + 1/Z + broadcast + y finalize for one
        # quarter. The last quarter finalizes on DVE (faster per-op than
        # GPSIMD) since its tail is the only one not hidden by later compute.
        def emit_tail(i0, u_t, zaccA, zaccB, last=False):
            zt = zp.tile([128, 4], f32, tag="z", name="zt")
            for c in range(4):
                nc.tensor.matmul(
                    zt[:, c:c + 1],
                    lhsT=zaccA[:, 128 * c:128 * (c + 1)],
                    rhs=ones_sb[:],
                    start=True,
                    stop=False,
                )
                nc.tensor.matmul(
                    zt[:, c:c + 1],
                    lhsT=zaccB[:, 128 * c:128 * (c + 1)],
                    rhs=ones_sb[:],
                    start=False,
                    stop=True,
                )
            rt = rpool.tile([128, 4], f32, tag="rt", name="rt")
            nc.vector.reciprocal(rt[:], zt[:, 0:4])
            nc.vector.tensor_scalar_mul(rt[:], rt[:], gammac_sb[:, 0:1])
            rd_ps = zp.tile([1, 512], f32, tag="z", name="rd_ps")
            for c in range(4):
                nc.tensor.matmul(
                    rd_ps[0:1, 128 * c:128 * (c + 1)],
                    lhsT=rt[:, c:c + 1],
                    rhs=ident_sb[:],
                    start=True,
                    stop=True,
                )
            rd = rpool.tile([1, 512], f32, tag="rd", name="rd")
            nc.vector.tensor_copy(rd[:], rd_ps[0:1, :])
            rb_ps = zp.tile([128, 512], f32, tag="z", name="rb_ps")
            for c in range(4):
                nc.tensor.matmul(
                    rb_ps[:, 128 * c:128 * (c + 1)],
                    lhsT=onesr_sb[:],
                    rhs=rd[0:1, 128 * c:128 * (c + 1)],
                    start=True,
                    stop=True,
                )
            rb_sb = rpool.tile([128, 512], f32, tag="rb", name="rb_sb")
            nc.vector.tensor_copy(rb_sb[:], rb_ps[:, :])
            eng = nc.vector if last else nc.gpsimd
            for ct in range(2):
                yt = ypool.tile([128, 512], f32, tag="y", name="yt")
                eng.tensor_mul(yt[:], u_t[ct][:], rb_sb[:])
                eng.tensor_add(yt[:], yt[:], x_sb[ct][:, i0:i0 + 512])
                nc.sync.dma_start(
                    out=y_ap[ct * 128:(ct + 1) * 128, i0:i0 + 512], in_=yt[:]
                )

        for qd in range(4):
            i0 = qd * 512
            u_t = [
                up.tile([128, 512], f32, tag=f"u{ct}", name=f"u{ct}", bufs=1)
                for ct in range(2)
            ]
            zaccA = rpool.tile([128, 512], bf16, tag="zaccA")
            zaccB = rpool.tile([128, 512], bf16, tag="zaccB")
            for J in range(4):
                e_tiles = []
                for pair in range(2):
                    stp = ps.tile([128, 1024], f32, tag="ps")
                    for h in range(2):
                        g = 2 * pair + h
                        nc.tensor.matmul(
                            stp[:, h * 512:(h + 1) * 512],
                            lhsT=kT4[32 * g:32 * (g + 1), J * 128:(J + 1) * 128],
                            rhs=qT4[32 * g:32 * (g + 1), i0:i0 + 512],
                            start=True,
                            stop=True,
                            tile_position=(32 * g, 0),
                        )
                    e2 = epool.tile([128, 1024], bf16, tag="e")
                    nc.scalar.activation(e2[:], stp[:], EXP)
                    e_tiles.append(e2)
                for g in range(4):
                    jb = 4 * J + g
                    eh = e_tiles[g // 2][:, (g % 2) * 512:(g % 2 + 1) * 512]
                    for ct in range(2):
                        nc.tensor.matmul(
                            u_t[ct][:, :],
                            lhsT=vT_sb[:, jb * C + ct * 128:jb * C + ct * 128 + 128],
                            rhs=eh,
                            start=(jb == 0),
                            stop=(jb == 15),
                        )
                    ztgt = zaccA if jb < 8 else zaccB
                    if jb in (0, 8):
                        nc.vector.tensor_copy(ztgt[:], eh)
                    else:
                        nc.vector.tensor_add(ztgt[:], ztgt[:], eh)
            # free the U psum banks for the next quarter right away
            us = []
            for ct in range(2):
                u = uspool.tile([128, 512], f32, tag=f"us{ct}", name=f"us{ct}")
                nc.vector.tensor_copy(u[:], u_t[ct][:, :])
                us.append(u)
            emit_tail(i0, us, zaccA, zaccB, last=(qd == 3))

    nc.compile()
    return nc


def get_nc():
    if "nc" not in _cache:
        _cache["nc"] = _build_nc()
    return _cache["nc"]


def make_in_maps(x, Wq, bq, Wk, bk, Wv, bv, gamma):
    import ml_dtypes

    bf = ml_dtypes.bfloat16
    x = np.asarray(x, dtype=np.float32)
    g = float(np.asarray(gamma, np.float32).reshape(-1)[0])
    shared = {
        "wqk": np.ascontiguousarray(
            np.concatenate([np.asarray(Wq, np.float32).T,
                            np.asarray(Wk, np.float32).T], axis=1)).astype(bf),
        "wvT": np.ascontiguousarray(np.asarray(Wv, np.float32).T).astype(bf),
        "bqk": np.concatenate([np.asarray(bq, np.float32).reshape(D, 1),
                               np.asarray(bk, np.float32).reshape(D, 1)], axis=0),
        "bvr": np.broadcast_to(np.asarray(bv, np.float32)[None, :], (128, C)).copy(),
        "gammac": np.full((128, 1), g, np.float32),
        "ones": np.ones((128, 1), bf),
        "onesr": np.ones((1, 128), np.float32),
        "ident": np.eye(128, dtype=np.float32),
    }
    return [
        dict(shared, x=np.ascontiguousarray(x[b]), xb=np.ascontiguousarray(x[b]).astype(bf))
        for b in range(B)
    ]


def kernel(x, Wq, bq, Wk, bk, Wv, bv, gamma):
    from concourse.bass_utils import run_bass_kernel_spmd

    nc = get_nc()
    in_maps = make_in_maps(x, Wq, bq, Wk, bk, Wv, bv, gamma)
    res = run_bass_kernel_spmd(nc, in_maps, list(range(NCORES)))
    return np.stack([res.results[b]["y"] for b in range(B)], axis=0)


# revision 24
# speedup vs baseline: 1.0443x; 1.0443x over previous
"""Trainium2 Bass kernel for nn_AttentionModule (B=8, C=256, L=2048, D=32).

Per-batch computation (data-parallel: one batch per NeuronCore, 8 cores):
    qT = Wq @ x + bq            # (D, L)
    kT = Wk @ x + bk            # (D, L)
    vT = x.T @ Wv.T + bv        # (L, C)   -- v transposed, computed directly
    ST = kT.T @ qT              # (L_j, L_i) = S[i,j] transposed
    E  = exp(ST)                # no max-subtraction: max|S| ~ 46, exp fits fp32
    Z  = sum_j E[j, i]          # DVE accumulate; partition-reduce on the PE
    U  = vT.T @ E               # (C, L_i)
    y  = gamma * U / Z + x

Engine split: PE does all matmuls incl. Z partition-reduction, the Z
transpose (so the reciprocal runs 128-wide instead of on one lane), and
the 1/Z broadcast. ACT does exp + q/k copyback. DVE does Z accumulation,
vT copyback, U copyback, reciprocal. GPSIMD does the final
y = U*(gamma/Z) + x elementwise work. Score matmuls (K=32) are 4x
row-packed via tile_position. Weights are transposed / replicated / cast
host-side.
"""

import numpy as np

B, C, L, D = 8, 256, 2048, 32
NCORES = 8

_cache = {}


def _build_nc():
    from contextlib import ExitStack

    import concourse.bacc as bacc
    import concourse.tile as tile
    from concourse import mybir

    f32 = mybir.dt.float32
    bf16 = mybir.dt.bfloat16
    EXP = mybir.ActivationFunctionType.Exp
    IDENT = mybir.ActivationFunctionType.Identity

    nc = bacc.Bacc("TRN2", target_bir_lowering=False, debug=False)

    x_d = nc.dram_tensor("x", [C, L], f32, kind="ExternalInput")
    xb_d = nc.dram_tensor("xb", [C, L], bf16, kind="ExternalInput")
    wqk_d = nc.dram_tensor("wqk", [C, 2 * D], bf16, kind="ExternalInput")
    wvT_d = nc.dram_tensor("wvT", [C, C], bf16, kind="ExternalInput")
    bqk_d = nc.dram_tensor("bqk", [2 * D, 1], f32, kind="ExternalInput")
    bvr_d = nc.dram_tensor("bvr", [128, C], f32, kind="ExternalInput")
    gammac_d = nc.dram_tensor("gammac", [128, 1], f32, kind="ExternalInput")
    ones_d = nc.dram_tensor("ones", [128, 1], bf16, kind="ExternalInput")
    onesr_d = nc.dram_tensor("onesr", [1, 128], f32, kind="ExternalInput")
    ident_d = nc.dram_tensor("ident", [128, 128], f32, kind="ExternalInput")
    y_d = nc.dram_tensor("y", [C, L], f32, kind="ExternalOutput")

    x_ap = x_d.ap()
    y_ap = y_d.ap()

    with tile.TileContext(nc) as tc, ExitStack() as ctx:
        singles = ctx.enter_context(tc.tile_pool(name="singles", bufs=1))
        big = ctx.enter_context(tc.tile_pool(name="big", bufs=1))
        ps = ctx.enter_context(tc.tile_pool(name="ps", bufs=2, space="PSUM"))
        up = ctx.enter_context(tc.tile_pool(name="up", bufs=1, space="PSUM"))
        zp = ctx.enter_context(tc.tile_pool(name="zp", bufs=2, space="PSUM"))
        epool = ctx.enter_context(tc.tile_pool(name="epool", bufs=4))
        ypool = ctx.enter_context(tc.tile_pool(name="ypool", bufs=4))
        uspool = ctx.enter_context(tc.tile_pool(name="uspool", bufs=2))
        rpool = ctx.enter_context(tc.tile_pool(name="rpool", bufs=2))

        # ---- load compute inputs first (xb then weights); fp32 x comes later ----
        xb_sb = []
        for ct in range(2):
            tb = big.tile([128, L], bf16, tag=f"xb{ct}")
            nc.sync.dma_start(out=tb[:], in_=xb_d.ap()[ct * 128:(ct + 1) * 128, :])
            xb_sb.append(tb)
        wqk_sb, wvT_sb = [], []
        for ct in range(2):
            tq = singles.tile([128, 2 * D], bf16, tag=f"wqk{ct}")
            nc.sync.dma_start(out=tq[:], in_=wqk_d.ap()[ct * 128:(ct + 1) * 128, :])
            wqk_sb.append(tq)
            tv = singles.tile([128, C], bf16, tag=f"wv{ct}")
            nc.sync.dma_start(out=tv[:], in_=wvT_d.ap()[ct * 128:(ct + 1) * 128, :])
            wvT_sb.append(tv)
        bqk_sb = singles.tile([2 * D, 1], f32, tag="bqk")
        nc.sync.dma_start(out=bqk_sb[:], in_=bqk_d.ap()[:, :])
        bvr_sb = singles.tile([128, C], f32, tag="bvr")
        nc.sync.dma_start(out=bvr_sb[:], in_=bvr_d.ap()[:, :])
        gammac_sb = singles.tile([128, 1], f32, tag="gammac")
        nc.sync.dma_start(out=gammac_sb[:], in_=gammac_d.ap()[:, :])
        ones_sb = singles.tile([128, 1], bf16, tag="ones")
        nc.sync.dma_start(out=ones_sb[:], in_=ones_d.ap()[:, :])
        onesr_sb = singles.tile([1, 128], f32, tag="onesr")
        nc.sync.dma_start(out=onesr_sb[:], in_=onesr_d.ap()[:, :])
        ident_sb = singles.tile([128, 128], f32, tag="ident")
        nc.sync.dma_start(out=ident_sb[:], in_=ident_d.ap()[:, :])

        # ---- projections ----
        # qT4: qT[d, i] replicated across the four 32-partition strips so the
        # row-packed score matmuls can stream the moving operand from any strip.
        qT4 = big.tile([128, L], bf16, tag="qT4")
        kT64 = big.tile([2 * D, L], bf16, tag="kT64")
        for it in range(4):
            p = ps.tile([128, 1024], f32, tag="ps")
            for ct in range(2):
                nc.tensor.matmul(
                    p[:2 * D, 0:512],
                    lhsT=wqk_sb[ct][:],
                    rhs=xb_sb[ct][:, it * 512:(it + 1) * 512],
                    start=(ct == 0),
                    stop=(ct == 1),
                )
            nc.scalar.activation(
                qT4[0:D, it * 512:(it + 1) * 512], p[0:D, 0:512], IDENT,
                bias=bqk_sb[0:D, :],
            )
            nc.scalar.activation(
                kT64[D:2 * D, it * 512:(it + 1) * 512], p[D:2 * D, 0:512], IDENT,
                bias=bqk_sb[D:2 * D, :],
            )
        for g in range(1, 4):
            nc.sync.dma_start(out=qT4[32 * g:32 * (g + 1), :], in_=qT4[0:D, :])
        # kT4: strip g holds kT j-blocks {4J+g}; round J lives at free cols J*128.
        kT4 = big.tile([128, 512], bf16, tag="kT4")
        for g in range(4):
            nc.sync.dma_start(
                out=kT4[32 * g:32 * (g + 1), :].rearrange("d (J j) -> d J j", j=128),
                in_=kT64[D:2 * D, :].rearrange("d (J G j) -> d J G j", G=4, j=128)[:, :, g, :],
            )

        # vT[j, c] stored as [128, 16*256]: block jb holds vT[jb*128 + p, c]
        vT_sb = big.tile([128, 16 * C], bf16, tag="vT")
        for lb in range(16):
            p = ps.tile([128, 1024], f32, tag="ps")
            for ct in range(2):
                nc.tensor.matmul(
                    p[:, :C],
                    lhsT=xb_sb[ct][:, lb * 128:(lb + 1) * 128],
                    rhs=wvT_sb[ct][:],
                    start=(ct == 0),
                    stop=(ct == 1),
                )
            nc.vector.tensor_add(
                vT_sb[:, lb * C:(lb + 1) * C], p[:, :C], bvr_sb[:]
            )

        # fp32 x, needed only for the residual at the end
        x_sb = []
        for ct in range(2):
            t = big.tile([128, L], f32, tag=f"x{ct}")
            nc.sync.dma_start(out=t[:], in_=x_ap[ct * 128:(ct + 1) * 128, :])
            x_sb.append(t)

        # ---- attention, processed in i-quarters of 512 columns ----
        # emit_tail: Z partition-reduce + 1/Z + broadcast + y finalize for one
        # quarter. The last quarter finalizes on DVE (faster per-op than
        # GPSIMD) since its tail is the only one not hidden by later compute.
        def emit_tail(i0, u_t, zaccA, zaccB, last=False):
            zt = zp.tile([128, 4], f32, tag="z", name="zt")
            for c in range(4):
                nc.tensor.matmul(
                    zt[:, c:c + 1],
                    lhsT=zaccA[:, 128 * c:128 * (c + 1)],
                    rhs=ones_sb[:],
                    start=True,
                    stop=False,
                )
                nc.tensor.matmul(
                    zt[:, c:c + 1],
                    lhsT=zaccB[:, 128 * c:128 * (c + 1)],
                    rhs=ones_sb[:],
                    start=False,
                    stop=True,
                )
            rt = rpool.tile([128, 4], f32, tag="rt", name="rt")
            nc.vector.reciprocal(rt[:], zt[:, 0:4])
            nc.vector.tensor_scalar_mul(rt[:], rt[:], gammac_sb[:, 0:1])
            rd_ps = zp.tile([1, 512], f32, tag="z", name="rd_ps")
            for c in range(4):
                nc.tensor.matmul(
                    rd_ps[0:1, 128 * c:128 * (c + 1)],
                    lhsT=rt[:, c:c + 1],
                    rhs=ident_sb[:],
                    start=True,
                    stop=True,
                )
            rd = rpool.tile([1, 512], f32, tag="rd", name="rd")
            nc.vector.tensor_copy(rd[:], rd_ps[0:1, :])
            rb_ps = zp.tile([128, 512], f32, tag="z", name="rb_ps")
            for c in range(4):
                nc.tensor.matmul(
                    rb_ps[:, 128 * c:128 * (c + 1)],
                    lhsT=onesr_sb[:],
                    rhs=rd[0:1, 128 * c:128 * (c + 1)],
                    start=True,
                    stop=True,
                )
            rb_sb = rpool.tile([128, 512], f32, tag="rb", name="rb_sb")
            nc.vector.tensor_copy(rb_sb[:], rb_ps[:, :])
            eng = nc.vector if last else nc.gpsimd
            for ct in range(2):
                yt = ypool.tile([128, 512], f32, tag="y", name="yt")
                eng.tensor_mul(yt[:], u_t[ct][:], rb_sb[:])
                eng.tensor_add(yt[:], yt[:], x_sb[ct][:, i0:i0 + 512])
                nc.sync.dma_start(
                    out=y_ap[ct * 128:(ct + 1) * 128, i0:i0 + 512], in_=yt[:]
                )

        for qd in range(4):
            i0 = qd * 512
            u_t = [
                up.tile([128, 512], f32, tag=f"u{ct}", name=f"u{ct}", bufs=1)
                for ct in range(2)
            ]
            zaccA = rpool.tile([128, 512], bf16, tag="zaccA")
            zaccB = rpool.tile([128, 512], bf16, tag="zaccB")
            for J in range(4):
                e_tiles = []
                for pair in range(2):
                    stp = ps.tile([128, 1024], f32, tag="ps")
                    for h in range(2):
                        g = 2 * pair + h
                        nc.tensor.matmul(
                            stp[:, h * 512:(h + 1) * 512],
                            lhsT=kT4[32 * g:32 * (g + 1), J * 128:(J + 1) * 128],
                            rhs=qT4[32 * g:32 * (g + 1), i0:i0 + 512],
                            start=True,
                            stop=True,
                            tile_position=(32 * g, 0),
                        )
                    e2 = epool.tile([128, 1024], bf16, tag="e")
                    nc.scalar.activation(e2[:], stp[:], EXP)
                    e_tiles.append(e2)
                for g in range(4):
                    jb = 4 * J + g
                    eh = e_tiles[g // 2][:, (g % 2) * 512:(g % 2 + 1) * 512]
                    for ct in range(2):
                        nc.tensor.matmul(
                            u_t[ct][:, :],
                            lhsT=vT_sb[:, jb * C + ct * 128:jb * C + ct * 128 + 128],
                            rhs=eh,
                            start=(jb == 0),
                            stop=(jb == 15),
                        )
                    ztgt = zaccA if jb % 2 == 0 else zaccB
                    if jb in (0, 1):
                        nc.vector.tensor_copy(ztgt[:], eh)
                    else:
                        nc.vector.tensor_add(ztgt[:], ztgt[:], eh)
            # free the U psum banks for the next quarter right away
            us = []
            for ct in range(2):
                u = uspool.tile([128, 512], f32, tag=f"us{ct}", name=f"us{ct}")
                nc.vector.tensor_copy(u[:], u_t[ct][:, :])
                us.append(u)
            emit_tail(i0, us, zaccA, zaccB, last=(qd == 3))

    nc.compile()
    return nc


def get_nc():
    if "nc" not in _cache:
        _cache["nc"] = _build_nc()
    return _cache["nc"]


def make_in_maps(x, Wq, bq, Wk, bk, Wv, bv, gamma):
    import ml_dtypes

    bf = ml_dtypes.bfloat16
    x = np.asarray(x, dtype=np.float32)
    g = float(np.asarray(gamma, np.float32).reshape(-1)[0])
    shared = {
        "wqk": np.ascontiguousarray(
            np.concatenate([np.asarray(Wq, np.float32).T,
                            np.asarray(Wk, np.float32).T], axis=1)).astype(bf),
        "wvT": np.ascontiguousarray(np.asarray(Wv, np.float32).T).astype(bf),
        "bqk": np.concatenate([np.asarray(bq, np.float32).reshape(D, 1),
                               np.asarray(bk, np.float32).reshape(D, 1)], axis=0),
        "bvr": np.broadcast_to(np.asarray(bv, np.float32)[None, :], (128, C)).copy(),
        "gammac": np.full((128, 1), g, np.float32),
        "ones": np.ones((128, 1), bf),
        "onesr": np.ones((1, 128), np.float32),
        "ident": np.eye(128, dtype=np.float32),
    }
    return [
        dict(shared, x=np.ascontiguousarray(x[b]), xb=np.ascontiguousarray(x[b]).astype(bf))
        for b in range(B)
    ]


def kernel(x, Wq, bq, Wk, bk, Wv, bv, gamma):
    from concourse.bass_utils import run_bass_kernel_spmd

    nc = get_nc()
    in_maps = make_in_maps(x, Wq, bq, Wk, bk, Wv, bv, gamma)
    res = run_bass_kernel_spmd(nc, in_maps, list(range(NCORES)))
    return np.stack([res.results[b]["y"] for b in range(B)], axis=0)
